# revision 16
# baseline (speedup 1.0000x reference)
"""Trainium2 Bass kernel for nn_MoE_32332513804634.

MoE: 16 routed experts (top-6, softmax-then-bias routing) + dense shared
expert, T=4096 tokens, D=2048, H=1408, HS=2816, fp32.

Strategy (8 NeuronCores, SPMD):
  - Host computes the gate (cheap) and per-expert token lists.
  - Expert parallelism as a per-core list of variable-width token chunks
    (width compiled in, identical multiset on every core; each chunk binds
    one expert's weights via its own dram tensors). Experts are cut into
    near-equal pieces and pieces are sorted+grouped 8-at-a-time into slots
    so the compiled capacity is within ~3% of the 3072/core lower bound
    (vs ~33% padding waste for fixed 2048/512 slot caps).
  - Each chunk runs SwiGLU with bf16 matmuls accumulating in fp32 PSUM,
    the per-token combine weight applied as a per-partition DVE scale on
    the PSUM->SBUF copy.  Outputs are written in bf16 (halves output DMA).
  - Shared expert is tensor-parallel over its 2816 hidden dim (352 rows
    per core, padded to 384); its weights are SBUF-resident (loaded once,
    reused by all 8 token chunks).  Shared chunks are interleaved between
    routed chunks to smooth DMA pressure.
  - Host scatters chunk outputs back to token rows, sums partials, and
    adds the second-layer biases (cw*b2 per expert, bs2 once) in fp32.
"""

import sys
import numpy as np

sys.path.insert(0, "/opt/trn_rl_repo")

import concourse.bass as bass  # noqa: E402
import concourse.tile as tile  # noqa: E402
from concourse import bacc, mybir  # noqa: E402
from concourse.bass_utils import run_bass_kernel_spmd  # noqa: E402

T = 4096
D = 2048
H = 1408
E = 16
TOP_K = 6
HS = 2816
N_CORES = 8
HM = H // 128          # 11
KO = D // 128          # 16
HS_PAD = 384           # shared hidden shard (352) padded to 3*128
HMS = HS_PAD // 128    # 3
F32 = mybir.dt.float32
BF16 = mybir.dt.bfloat16
MM_DT = BF16

_PROGRAM_CACHE: dict = {}


def _to_mm(a):
    import ml_dtypes
    return np.ascontiguousarray(a).astype(ml_dtypes.bfloat16)


def _host_gate(xf, gate_w, gate_b):
    scores = xf @ gate_w.T
    m = scores.max(axis=-1, keepdims=True)
    p = np.exp(scores - m, dtype=np.float32)
    probs = p / p.sum(axis=-1, keepdims=True)
    biased = probs + gate_b
    idx = np.argpartition(biased, E - TOP_K, axis=1)[:, E - TOP_K:]
    mask = np.zeros((xf.shape[0], E), dtype=bool)
    mask[np.arange(xf.shape[0])[:, None], idx] = True
    cw = np.where(mask, probs, 0.0).astype(np.float32)
    toks = [np.flatnonzero(mask[:, e]).astype(np.int64) for e in range(E)]
    return cw, toks


def _chunk_cost(w):
    """Approx PE cost (ns) of one compiled chunk of width w."""
    l1 = 11 * 16 * 2 * max(107.0, w / 2.4 + 16)
    l2 = 4 * ((w + 127) // 128) * 11 * (512 / 2.4 + 16)
    return l1 + l2


def _cut_pieces(counts, target):
    """Cut each expert into near-equal pieces (each <= 512)."""
    pieces = []
    for e, c in enumerate(counts):
        c = int(c)
        if c == 0:
            continue
        k = max(1, -(-c // target))
        while -(-c // k) > 512:
            k += 1
        base, rem = divmod(c, k)
        start = 0
        for i in range(k):
            n = base + (1 if i < rem else 0)
            pieces.append((n, e, start))
            start += n
    return pieces


def _cut_pieces_base(counts, base_sz):
    """Cut into pieces of base_sz plus one ragged final piece per expert."""
    pieces = []
    for e, c in enumerate(counts):
        c = int(c)
        start = 0
        while c >= base_sz + 128:
            pieces.append((base_sz, e, start))
            start += base_sz
            c -= base_sz
        if c > 512:
            h1 = (c + 1) // 2
            pieces.append((h1, e, start))
            start += h1
            c -= h1
        if c > 0:
            pieces.append((c, e, start))
    return pieces


def _plan_groupsort(counts):
    """Equal-cut pieces, sorted and grouped 8-at-a-time into slots."""
    best = None
    cand = [_cut_pieces(counts, t) for t in range(320, 513, 8)]
    cand += [_cut_pieces_base(counts, b) for b in (512, 448, 384)]
    for pieces in cand:
        ps = sorted(pieces, key=lambda p: -p[0])
        nslots = -(-len(ps) // N_CORES)
        widths = []
        for s in range(nslots):
            grp = ps[s * N_CORES:(s + 1) * N_CORES]
            w = -(-max(p[0] for p in grp) // 16) * 16
            widths.append(w)
        cost = sum(_chunk_cost(w) for w in widths)
        if best is None or cost < best[0]:
            best = (cost, tuple(widths), ps)
    cost, widths, ps = best
    assignment = [[None] * len(widths) for _ in range(N_CORES)]
    for i, (n, e, st) in enumerate(ps):
        s, c = divmod(i, N_CORES)
        assignment[c][s] = (e, st, n)
    return cost, widths, assignment


def _solve_bundles(nz, W1, W2, I1, I2):
    """Exact DP: pick one (i, j) bundle per expert with sum(i) <= I1,
    sum(j) <= I2. Returns list of (waste, i, j) per expert or None."""
    opts = []
    for e, c in nz:
        o = []
        for i in range(0, min(I1, -(-c // W1)) + 1):
            rem = c - i * W1
            j = max(0, -(-rem // W2)) if W2 > 0 else 0
            if j > I2 or (W2 == 0 and rem > 0):
                continue
            o.append((i * W1 + j * W2 - c, i, j))
        if not o:
            return None
        opts.append(o)
    reach = [np.zeros((I1 + 1, I2 + 1), dtype=bool)]
    reach[0][0, 0] = True
    for o in opts:
        cur = reach[-1]
        nxt = np.zeros_like(cur)
        for _, i, j in o:
            if i <= I1 and j <= I2:
                nxt[i:, j:] |= cur[:I1 + 1 - i, :I2 + 1 - j]
        if not nxt.any():
            return None
        reach.append(nxt)
    # backtrack from any reachable final state (prefer min waste greedily)
    si, sj = np.argwhere(reach[-1])[0]
    pick = [None] * len(opts)
    for idx in range(len(opts) - 1, -1, -1):
        cands = sorted(opts[idx])
        for w, i, j in cands:
            if i <= si and j <= sj and reach[idx][si - i, sj - j]:
                pick[idx] = (w, i, j)
                si, sj = si - i, sj - j
                break
        if pick[idx] is None:
            return None
    return pick


def _plan_twowidth(counts):
    """Per-core (a x W1 + b x W2) slots; experts assigned (i, j) slot
    bundles via exact DP; configs tried in ascending PE-cost order."""
    nz = [(e, int(c)) for e, c in enumerate(counts) if c > 0]
    total = sum(c for _, c in nz)
    configs = []
    for W1, W2 in ((512, 448), (512, 384), (512, 320), (512, 256),
                   (448, 384), (512, 0)):
        for a in range(0, 9):
            for b in range(0, 11 if W2 else 1):
                cap = a * W1 + b * W2
                if cap * N_CORES < total or cap > 4608:
                    continue
                cost = a * _chunk_cost(W1) + b * _chunk_cost(W2)
                configs.append((cost, W1, W2, a, b))
    configs.sort()
    best = None
    for cost, W1, W2, a, b in configs:
        pick = _solve_bundles(nz, W1, W2, a * N_CORES, b * N_CORES)
        if pick is not None:
            best = (cost, (W1, W2, a, b), pick,
                    [(e, c, None) for e, c in nz])
            break
    if best is None:
        return None
    cost, (W1, W2, a, b), pick, opts = best
    widths = (W1,) * a + (W2,) * b
    # build pieces per expert: i full-ish W1 pieces then j W2 pieces
    slots1 = [(c, s) for s in range(a) for c in range(N_CORES)]
    slots2 = [(c, s + a) for s in range(b) for c in range(N_CORES)]
    assignment = [[None] * len(widths) for _ in range(N_CORES)]
    i1 = i2 = 0
    for (e, c, _), (w, i, j) in zip(opts, pick):
        start = 0
        rem = c
        for k in range(i + j):
            size = W1 if k < i else W2
            if k < i:
                core, s = slots1[i1]; i1 += 1
            else:
                core, s = slots2[i2]; i2 += 1
            n = min(rem, size)
            if n > 0:
                assignment[core][s] = (e, start, n)
            start += n
            rem -= n
        assert rem == 0, (e, c, rem)
    return cost, widths, assignment


def _plan(counts):
    """Returns (widths, assignment): widths = per-core compiled chunk
    widths; assignment[core][slot] = (expert, start, fill) or None."""
    plans = [_plan_groupsort(counts)]
    tw = _plan_twowidth(counts)
    if tw is not None:
        plans.append(tw)
    plans.sort(key=lambda p: p[0])
    _, widths, assignment = plans[0]
    return tuple(widths), assignment


def _build_program(widths):
    nc = bacc.Bacc("TRN2", debug=False, num_devices=N_CORES)

    ins = {}
    outs = {}

    def din(name, shape, dt=MM_DT):
        ins[name] = nc.dram_tensor(name, list(shape), dt, kind="ExternalInput").ap()
        return ins[name]

    def dout(name, shape, dt=BF16):
        outs[name] = nc.dram_tensor(name, list(shape), dt, kind="ExternalOutput").ap()
        return outs[name]

    for s, w in enumerate(widths):
        ntch = -(-w // 128)
        din(f"xg{s}", (D, w))
        din(f"w1t{s}", (D, H))
        din(f"w3t{s}", (D, H))
        din(f"w2ta{s}", (H, D))
        din(f"b1_{s}", (128, HM), F32)
        din(f"b3_{s}", (128, HM), F32)
        din(f"scl{s}", (128, ntch), F32)
        dout(f"oe{s}", (ntch * 128, D))
    # shared expert: token-sharded (512 tokens/core, full 2816 hidden)
    HMS22 = HS // 128  # 22
    din("xs", (D, 512))
    din("ws1t", (D, HS))
    din("ws3t", (D, HS))
    din("ws2ta", (HS, D))
    din("bs1", (128, HMS22), F32)
    din("bs3", (128, HMS22), F32)
    dout("zs", (512, D))

    with tile.TileContext(nc) as tc:
        with (
            tc.tile_pool(name="xpool", bufs=2) as xpool,
            tc.tile_pool(name="hpool", bufs=2) as hpool,
            tc.tile_pool(name="wcol", bufs=3) as wcol,
            tc.tile_pool(name="w2pool", bufs=2) as w2pool,
            tc.tile_pool(name="tmp", bufs=2) as tmp,
            tc.tile_pool(name="opool", bufs=4) as opool,
            tc.tile_pool(name="cpool", bufs=1) as cpool,
            tc.tile_pool(name="pp", bufs=2, space="PSUM") as pp,
        ):
            def mlp_chunk(xg_ap, w1_ap, w3_ap, w2_ap, b1_ap, b3_ap, scl_ap,
                          out_ap, w, n_hm, wtag):
                """One chunk: out[:w] = scale * (swiglu(xg) @ W2^T)."""
                ntch = -(-w // 128)
                x3 = xg_ap.rearrange("(ko p) t -> p ko t", p=128)
                w1c3 = w1_ap.rearrange("(ko p) h -> p ko h", p=128)
                w3c3 = w3_ap.rearrange("(ko p) h -> p ko h", p=128)
                w23 = w2_ap.rearrange("(k p) d -> p k d", p=128)

                b1sb = cpool.tile([128, n_hm], F32, tag=f"b1{wtag}")
                nc.sync.dma_start(b1sb[:], b1_ap)
                b3sb = cpool.tile([128, n_hm], F32, tag=f"b3{wtag}")
                nc.sync.dma_start(b3sb[:], b3_ap)
                if scl_ap is not None:
                    sclsb = cpool.tile([128, ntch], F32, tag=f"scl{wtag}")
                    nc.sync.dma_start(sclsb[:], scl_ap)

                xsb = xpool.tile([128, KO, 512], MM_DT, tag="xg")
                nc.sync.dma_start(xsb[:, :, :w], x3)
                nw2 = 4 if n_hm == HM else 2
                w2sbs = []
                hsb = hpool.tile([128, n_hm, 512], MM_DT, tag=f"h{n_hm}",
                                 bufs=(2 if n_hm == HM else 1))
                for hm in range(n_hm):
                    if hm == 2:
                        # prefetch W2 tiles once L1-critical DMAs are queued;
                        # they stream during the rest of L1
                        for dm in range(4):
                            w2sb = w2pool.tile([128, n_hm, 512], MM_DT,
                                               tag=f"w2s{n_hm}", bufs=nw2)
                            nc.sync.dma_start(
                                w2sb[:], w23[:, :, dm * 512:(dm + 1) * 512])
                            w2sbs.append(w2sb)
                    w1t_ = wcol.tile([128, KO, 128], MM_DT, tag="w1c")
                    nc.sync.dma_start(w1t_[:], w1c3[:, :, hm * 128:(hm + 1) * 128])
                    w3t_ = wcol.tile([128, KO, 128], MM_DT, tag="w3c")
                    nc.sync.dma_start(w3t_[:], w3c3[:, :, hm * 128:(hm + 1) * 128])
                    ps1 = pp.tile([128, 512], F32, tag="ph1")
                    for ko in range(KO):
                        nc.tensor.matmul(ps1[:, :w], w1t_[:, ko, :], xsb[:, ko, :w],
                                         start=(ko == 0), stop=(ko == KO - 1))
                    ps3 = pp.tile([128, 512], F32, tag="ph3")
                    for ko in range(KO):
                        nc.tensor.matmul(ps3[:, :w], w3t_[:, ko, :], xsb[:, ko, :w],
                                         start=(ko == 0), stop=(ko == KO - 1))
                    h1t = tmp.tile([128, 512], F32, tag="h1t")
                    nc.scalar.activation(h1t[:, :w], ps1[:, :w],
                                         mybir.ActivationFunctionType.Silu,
                                         bias=b1sb[:, hm:hm + 1])
                    h3t = tmp.tile([128, 512], F32, tag="h3t")
                    nc.scalar.activation(h3t[:, :w], ps3[:, :w],
                                         mybir.ActivationFunctionType.Identity,
                                         bias=b3sb[:, hm:hm + 1])
                    nc.vector.tensor_mul(hsb[:, hm, :w], h1t[:, :w], h3t[:, :w])
                # second matmul: out rows = tokens
                for dm in range(4):
                    w2sb = w2sbs[dm]
                    for tch in range(ntch):
                        tok0 = tch * 128
                        tcw = min(128, w - tok0)
                        ps2 = pp.tile([128, 512], F32, tag="po", bufs=4)
                        for k in range(n_hm):
                            lhsT = hsb[:, k, tok0:tok0 + tcw]
                            nc.tensor.matmul(ps2[:tcw, :], lhsT, w2sb[:, k, :],
                                             start=(k == 0), stop=(k == n_hm - 1))
                        osb = opool.tile([128, 512], BF16, tag="osb")
                        if scl_ap is not None:
                            nc.vector.tensor_scalar_mul(
                                osb[:tcw, :], ps2[:tcw, :], sclsb[:tcw, tch:tch + 1])
                        else:
                            nc.vector.tensor_copy(osb[:tcw, :], ps2[:tcw, :])
                        nc.sync.dma_start(
                            out_ap[tok0:tok0 + tcw, dm * 512:(dm + 1) * 512],
                            osb[:tcw, :])

            def routed_chunk(s, w):
                mlp_chunk(ins[f"xg{s}"], ins[f"w1t{s}"], ins[f"w3t{s}"],
                          ins[f"w2ta{s}"], ins[f"b1_{s}"], ins[f"b3_{s}"],
                          ins[f"scl{s}"], outs[f"oe{s}"], w, HM, f"e{s}")

            def shared_chunk():
                mlp_chunk(ins["xs"], ins["ws1t"], ins["ws3t"], ins["ws2ta"],
                          ins["bs1"], ins["bs3"], None, outs["zs"], 512,
                          HMS22, "sh")

            # shared chunk in the middle of the routed sequence
            n_r = len(widths)
            for i in range(n_r):
                routed_chunk(i, widths[i])
                if i == n_r // 2 - 1:
                    shared_chunk()

    nc.compile()
    return nc


def kernel(x, gate_w, gate_b, w1, b1, w2, b2, w3, b3,
           ws1, bs1, ws2, bs2, ws3, bs3):
    x = np.asarray(x, np.float32)
    xf = np.ascontiguousarray(x.reshape(-1, D))
    gate_w = np.asarray(gate_w, np.float32)
    gate_b = np.asarray(gate_b, np.float32)
    w1 = np.asarray(w1, np.float32)
    b1 = np.asarray(b1, np.float32)
    w2 = np.asarray(w2, np.float32)
    b2 = np.asarray(b2, np.float32)
    w3 = np.asarray(w3, np.float32)
    b3 = np.asarray(b3, np.float32)
    ws1 = np.asarray(ws1, np.float32)
    bs1 = np.asarray(bs1, np.float32)
    ws2 = np.asarray(ws2, np.float32)
    bs2 = np.asarray(bs2, np.float32)
    ws3 = np.asarray(ws3, np.float32)
    bs3 = np.asarray(bs3, np.float32)

    cw, toks = _host_gate(xf, gate_w, gate_b)
    counts = np.array([len(t) for t in toks])
    widths, assignment = _plan(counts)

    if widths not in _PROGRAM_CACHE:
        _PROGRAM_CACHE[widths] = _build_program(widths)
    nc = _PROGRAM_CACHE[widths]

    xT = np.ascontiguousarray(xf.T)  # [D, T]
    xT_mm = _to_mm(xT)

    w1t = {}
    w3t = {}
    w2ta = {}
    b1t = {}
    b3t = {}
    need = sorted({p[0] for slots in assignment for p in slots if p is not None})
    for e in need:
        w1t[e] = _to_mm(w1[e].T)
        w3t[e] = _to_mm(w3[e].T)
        w2ta[e] = _to_mm(w2[e].T)
        b1t[e] = np.ascontiguousarray(b1[e].reshape(HM, 128).T)
        b3t[e] = np.ascontiguousarray(b3[e].reshape(HM, 128).T)

    HMS22 = HS // 128  # 22
    ws1t = _to_mm(ws1.T)
    ws3t = _to_mm(ws3.T)
    ws2ta = _to_mm(ws2.T)
    bs1t = np.ascontiguousarray(bs1.reshape(HMS22, 128).T)
    bs3t = np.ascontiguousarray(bs3.reshape(HMS22, 128).T)

    in_maps = []
    for c in range(N_CORES):
        m = {}
        for s, w in enumerate(widths):
            ntch = -(-w // 128)
            piece = assignment[c][s]
            xg = np.zeros((D, w), np.float32)
            scl = np.zeros(ntch * 128, np.float32)
            if piece is None:
                e = need[0]
            else:
                e, s0, n = piece
                tk = toks[e][s0:s0 + n]
                xg[:, :n] = xT[:, tk]
                scl[:n] = cw[tk, e]
            m[f"w1t{s}"] = w1t[e]
            m[f"w3t{s}"] = w3t[e]
            m[f"w2ta{s}"] = w2ta[e]
            m[f"b1_{s}"] = b1t[e]
            m[f"b3_{s}"] = b3t[e]
            m[f"xg{s}"] = _to_mm(xg)
            m[f"scl{s}"] = np.ascontiguousarray(scl.reshape(ntch, 128).T)
        # shared expert: this core's 512 tokens, full weights
        m["xs"] = np.ascontiguousarray(xT_mm[:, c * 512:(c + 1) * 512])
        m["ws1t"] = ws1t
        m["ws3t"] = ws3t
        m["ws2ta"] = ws2ta
        m["bs1"] = bs1t
        m["bs3"] = bs3t
        in_maps.append(m)

    res = run_bass_kernel_spmd(nc, in_maps, list(range(N_CORES)))

    y = np.zeros((T, D), np.float32)
    for c in range(N_CORES):
        for s, w in enumerate(widths):
            piece = assignment[c][s]
            if piece is None:
                continue
            e, s0, n = piece
            tk = toks[e][s0:s0 + n]
            y[tk] += res.results[c][f"oe{s}"][:n].astype(np.float32)
            y[tk] += cw[tk, e][:, None] * b2[e][None, :]
        y[c * 512:(c + 1) * 512] += res.results[c]["zs"].astype(np.float32)
    y += bs2[None, :]
    return y.reshape(x.shape).astype(np.float32)
